# revision 24
# baseline (speedup 1.0000x reference)
"""CARAFE-naive upsampling (N=4, C=256, H=W=64, k=5, g=4, s=2) on 8 TRN2
NeuronCores.

Strategy
--------
Sharding: core c <- (batch n = c//2, group-pair j = c%2). Each core owns 128
feature channels (2 of the 4 mask groups) of one batch image.

Compute: blocked im2col. The output is tiled into 4x8 source blocks; a
block's 25-tap neighborhood lives in an 8x12 source window (K=96). Per
(tile, group) ONE matmul computes every tap in a single pass:

    psum[(h',a,w,b), c] = sum_{(r,w'')} statT[(r,w''), (h',a,w,b)]
                                      * feat[(r,w''), c]

statT is the host-sheared mask tile (each column holds one output pixel's
25 taps placed at its window offsets; 96/25 = 3.8x inflation over raw
masks), shipped in f8e3m4 (4 mantissa bits keep rel err at 1.3e-2 while
halving mask bytes); feat is the host-im2col'd feature window in bf16
(shared by both groups, N=64 channel columns each). Every psum element
is real output: full [128, 512] PSUM banks drain with one contiguous
128-partition DVE/ACT copy per (row-block, group).

Dataflow: everything is SBUF-resident. All 16 load DMAs (2-row-block
slabs; the first block ships alone so matmuls start early) are queued up
front on the sync HWDGE ring, which nothing else shares -- a copy or
store queued behind a load issue would head-of-line block the pipeline.
Scalar runs the g=1 copies and issues the 2-block-batched stores right
after them (same-engine order avoids a cross-engine semaphore). Total
HBM traffic 10.45 MB/core (3.15 stat + 3.1 feat + 4.2 out) and 256
LDWEIGHTS+MATMUL pairs; the kernel sits essentially on the DMA roofline,
with the serialized PE stream (~30 us) hidden under the DMA window.
"""

import sys

import numpy as np
from numpy.lib.stride_tricks import sliding_window_view

for _p in ("/opt/trn_rl_repo", "/opt/pypackages"):
    if _p not in sys.path:
        sys.path.append(_p)

import ml_dtypes  # noqa: E402
from contextlib import ExitStack  # noqa: E402

import concourse.bass as bass  # noqa: E402
import concourse.tile as tile  # noqa: E402
from concourse import bacc, mybir  # noqa: E402
from concourse.bass_utils import run_bass_kernel_spmd  # noqa: E402

# Problem constants (hardcoded per harness contract)
N, C, H, W = 4, 256, 64, 64
NB = 16          # row blocks (4 source rows each)
NWP = 8          # col blocks (8 source cols each)
K = 96           # contraction = 8x12 source window
KP = 96          # no K padding: padding to 128 turns on FWL's LDWEIGHTS/
                 # MATMUL overlap, but shipping 2.15MB of zero rows costs
                 # more DMA time than the overlap saves
BF16 = ml_dtypes.bfloat16
F8E3 = ml_dtypes.float8_e3m4

_NC_CACHE = {}


def _build_bass():
    nc = bacc.Bacc()
    # K-major DRAM layouts, byte-identical to the SBUF tiles: any hb range
    # is one contiguous-per-partition DMA slab of any size
    stat_d = nc.declare_dram_parameter(
        "stat", [KP, NB, 2, NWP, 128], mybir.dt.float8e3, isOutput=False)
    feat_d = nc.declare_dram_parameter(
        "feat", [KP, NB, NWP, 128], mybir.dt.bfloat16, isOutput=False)
    out_d = nc.declare_dram_parameter(
        "out", [NB // 2, 128, 2, 2, NWP, 64], mybir.dt.bfloat16,
        isOutput=True)

    with tile.TileContext(nc) as tc, ExitStack() as ctx:
        sp = ctx.enter_context(tc.tile_pool(name="sp", bufs=1))
        fp = ctx.enter_context(tc.tile_pool(name="fp", bufs=1))
        pp = ctx.enter_context(tc.tile_pool(name="pp", bufs=8, space="PSUM"))
        op = ctx.enter_context(tc.tile_pool(name="op", bufs=6))

        # whole input resident in SBUF: stat 64KB/part, feat 32KB/part
        stat_sb = sp.tile([KP, NB, 2, NWP, 128], mybir.dt.float8e3,
                          name="stat_sb", tag="st")
        feat_sb = fp.tile([KP, NB, NWP, 128], mybir.dt.bfloat16,
                          name="feat_sb", tag="ft")
        # All loads ride the sync HWDGE ring in geometrically growing
        # slabs (1,1,2,4,8 row-blocks): fine slabs up front start the MM
        # stream early; coarse slabs give each of the 16 SDMA engines
        # >=64KB contiguous work, off the descriptor-overhead floor that
        # capped 390KB slabs at ~69% of per-engine bandwidth. Nothing else
        # may be queued between the loads: ring FIFO would stall it.
        for lo, hi in ((0, 1), (1, 2), (2, 4), (4, 8), (8, NB)):
            nc.sync.dma_start(out=stat_sb[:, lo:hi], in_=stat_d[:, lo:hi])
            nc.sync.dma_start(out=feat_sb[:, lo:hi], in_=feat_d[:, lo:hi])

        ots = []
        for hb in range(NB):
            if hb % 2 == 0:
                ot2 = op.tile([128, 2, 2, NWP, 64], mybir.dt.bfloat16,
                              name=f"o{hb}", tag="ot")
                ots.append(ot2)
            ot = ot2[:, hb % 2]
            for g in range(2):
                ps = pp.tile([128, NWP, 64], mybir.dt.float32,
                             name=f"p{hb}_{g}", tag="ps")
                for wbp in range(NWP):
                    nc.tensor.matmul(
                        out=ps[:, wbp, :],
                        lhsT=stat_sb[:, hb, g, wbp, :],
                        rhs=feat_sb[:, hb, wbp, 64 * g: 64 * g + 64],
                        start=True, stop=True,
                        skip_group_check=True,
                    )
                # drain the full bank with one contiguous 128-partition copy
                if g == 0:
                    nc.vector.tensor_copy(out=ot[:, g], in_=ps)
                else:
                    nc.scalar.copy(out=ot[:, g], in_=ps)
        # Stores ride the SAME sync ring, queued after all 20 load issues:
        # ring FIFO defers them until the loads have drained, so the load
        # phase runs pure-read (~390 GB/s vs ~290 mixed) and the matmul
        # stream it paces finishes sooner. Store p's copy-semaphores are
        # long satisfied by the time it reaches the ring head.
        for p in range(NB // 2):
            nc.sync.dma_start(out=out_d[p], in_=ots[p])

    nc.finalize()
    return nc


def _host_shards(features, masks):
    """Build per-core stat/feat arrays (bf16)."""
    in_maps = []
    for core in range(8):
        n, j = core // 2, core % 2
        f = features[n, 128 * j: 128 * (j + 1)]        # [128, 64, 64] f32
        m = masks[n, 50 * j: 50 * j + 50].reshape(2, 25, 128, 128)

        # feature im2col: feat[hb, (r,w''), wbp, c] = Fpad[c, 4hb+r, 8wbp+w'']
        fpad = np.pad(f, ((0, 0), (2, 2), (2, 2)))
        sw = sliding_window_view(fpad, (8, 12), axis=(1, 2))[:, ::4, ::8]
        feat = np.ascontiguousarray(
            sw.transpose(1, 3, 4, 2, 0)).reshape(NB, K, NWP, 128)

        # mask shear: stat[hb, (r,w''), g, wbp, (h',a,w,b)] holds tap
        # (di=r-h', dj=w''-w) of output pixel (2(4hb+h')+a, 2(8wbp+w)+b)
        mm = m.reshape(2, 5, 5, NB, 4, 2, NWP, 8, 2)  # g,di,dj,hb,h,a,wbp,w,b
        stat = np.zeros((NB, 8, 12, 2, NWP, 4, 2, 8, 2), np.float32)
        for di in range(5):
            for dj in range(5):
                for hp in range(4):
                    for w in range(8):
                        stat[:, hp + di, w + dj, :, :, hp, :, w, :] = \
                            mm[:, di, dj, :, hp, :, :, w, :].transpose(
                                1, 0, 3, 2, 4)
        stat = stat.reshape(NB, K, 2, NWP, 128)

        stat = np.ascontiguousarray(stat.transpose(1, 0, 2, 3, 4))
        feat = np.ascontiguousarray(feat.transpose(1, 0, 2, 3))
        in_maps.append({
            "stat": stat.astype(F8E3),
            "feat": feat.astype(BF16),
        })
    return in_maps


def kernel(features, masks, _trace=False):
    features = np.asarray(features, dtype=np.float32)
    masks = np.asarray(masks, dtype=np.float32)

    in_maps = _host_shards(features, masks)

    if "nc" not in _NC_CACHE:
        _NC_CACHE["nc"] = _build_bass()
    nc = _NC_CACHE["nc"]

    res = run_bass_kernel_spmd(nc, in_maps, list(range(8)), trace=_trace)
    kernel._last_result = res

    out = np.empty((N, C, 2 * H, 2 * W), np.float32)
    for core in range(8):
        n, j = core // 2, core % 2
        od = res.results[core]["out"].astype(np.float32)
        od = od.transpose(0, 2, 1, 3, 4, 5).reshape(NB, 128, 2, NWP, 64)
        od = od.reshape(NB, 4, 2, 8, 2, 2, NWP, 64)  # hb,h',a,w,b,g,wbp,cc
        od = od.transpose(5, 7, 0, 1, 2, 6, 3, 4)    # g,cc,hb,h',a,wbp,w,b
        out[n, 128 * j: 128 * (j + 1)] = od.reshape(128, 128, 128)
    return out
